# revision 5
# baseline (speedup 1.0000x reference)
"""Trainium2 Bass kernel for batched dot-product attention.

Problem: B=8, QL=KL=2048, D=1024 fp32.
  scores = Q @ K^T / sqrt(D) + attn_mask;  scores[pad] = -1e12
  W = softmax(scores, axis=-1);  ctx = W @ V
Returns (ctx, W) like the reference.

Sharding: pure data-parallel — one batch per NeuronCore, 8 cores.

Per-core kernel (SPMD program, same for all cores):
  - K, V loaded resident in SBUF; K transposed to [d, k] layout via PE
    transposes (fp32 has no DMA-transpose path).
  - Loop over 16 q-tiles of 128 rows:
      MM1  : S = Qt^T @ Kt  (float32r matmuls, N=512, accumulate over d)
      TTR  : S_sbuf = S_psum + bias_row (pad mask as -1e12 additive bias),
             fused row-max (DVE tensor_tensor_reduce)
      EXP  : E = exp(S - rowmax) fused row-sum (ACT activation accum_out)
      NORM : W = E * (1/rowsum)  (DVE tensor_scalar, in place)
      Wt   : PE-transpose W 128x128 blocks for MM2
      MM2  : ctx = Wt^T @ V    (float32r, accumulate over k)
  - Software-pipelined: MM2/transposes of tile i-1 are emitted after
    MM1/softmax of tile i so the PE never waits on the softmax chain.
"""

import os

import numpy as np

import concourse.bacc as bacc
import concourse.bass as bass
import concourse.mybir as mybir
import concourse.tile as tile
from concourse.bass_utils import run_bass_kernel_spmd
from concourse.masks import make_identity

F32 = mybir.dt.float32
F32R = mybir.dt.float32r
AF = mybir.ActivationFunctionType
ALU = mybir.AluOpType

P = 128
NEG_BIG = -1.0e12

N_CORES = 8
FULL_B, FULL_QL, FULL_KL, FULL_D = 8, 2048, 2048, 1024

# set to False to use plain fp32 matmuls (4x slower, exact) if fp32r
# accuracy turns out to be insufficient
USE_F32R = os.environ.get("ATTN_NO_F32R", "") == ""

LAST_RESULTS = None  # test harness introspection


MMDT = F32R if USE_F32R else F32


def _mm_cast(ap):
    return ap


def build_attention(QL, KL, D, with_mask2d):
    """Build the per-core SPMD Bass program.

    with_mask2d=False: inputs q,k,v,bias ([1,KL] additive row bias).
    with_mask2d=True : inputs q,k,v,mask2d ([QL,KL] additive mask =
                       attn_mask + pad bias, prepared on host).
    """
    inv_temp = float(1.0 / np.float32(np.sqrt(np.float32(D))))
    NQ = QL // P          # q tiles
    NKB = KL // P         # k blocks of 128
    ND = D // P           # d blocks of 128
    NKC = KL // 512       # k chunks of 512 (MM1 N dim)
    NDC = D // 512        # d chunks of 512 (MM2 N dim)
    assert QL % P == 0 and KL % 512 == 0 and D % 512 == 0

    nc = bacc.Bacc("TRN2", target_bir_lowering=False)

    qd = nc.declare_dram_parameter("q", [QL, D], F32, isOutput=False)
    kd = nc.declare_dram_parameter("k", [KL, D], F32, isOutput=False)
    vd = nc.declare_dram_parameter("v", [KL, D], F32, isOutput=False)
    if with_mask2d:
        md = nc.declare_dram_parameter("mask2d", [QL, KL], F32, isOutput=False)
    else:
        bd = nc.declare_dram_parameter("bias", [1, KL], F32, isOutput=False)
    ctxd = nc.declare_dram_parameter("ctx", [QL, D], F32, isOutput=True)
    wd = nc.declare_dram_parameter("w", [QL, KL], F32, isOutput=True)

    with tile.TileContext(nc) as tc:
        with (
            tc.tile_pool(name="const", bufs=1) as constp,
            tc.tile_pool(name="kv", bufs=1) as kvp,
            tc.tile_pool(name="psum_s", bufs=1, space="PSUM") as psum_s,
            tc.tile_pool(name="psum_c", bufs=1, space="PSUM") as psum_c,
            tc.tile_pool(name="psum_t", bufs=2, space="PSUM") as psum_t,
        ):
            ident = constp.tile([P, P], F32)
            make_identity(nc, ident)

            if not with_mask2d:
                bias_row = constp.tile([1, KL], F32)
                nc.sync.dma_start(out=bias_row, in_=bd[0:1, :])
                bias_b = constp.tile([P, KL], F32)
                nc.gpsimd.partition_broadcast(bias_b, bias_row)

            # K^T resident: kt[:, dblk*KL + kcol], partition = d within block
            kt = kvp.tile([P, ND * KL], MMDT)
            # V resident (natural): vsb[:, kblk*D + dcol], partition = k in blk
            vsb = kvp.tile([P, NKB * D], MMDT)
            kt_r = kt.rearrange("p (b k) -> p b k", b=ND)

            # ---- prologue: load V, load K and transpose into kt ----
            for j in range(NKB):
                nc.sync.dma_start(
                    out=vsb[:, j * D : (j + 1) * D], in_=vd[j * P : (j + 1) * P, :].bitcast(MMDT)
                )
            with tc.tile_pool(name="knat", bufs=3) as knatp:
                for j in range(NKB):
                    knat = knatp.tile([P, D], F32)
                    nc.sync.dma_start(out=knat, in_=kd[j * P : (j + 1) * P, :])
                    for g in range(ND // 4):
                        tp = psum_t.tile([P, 512], F32, tag="tp")
                        for c in range(4):
                            dblk = g * 4 + c
                            nc.tensor.transpose(
                                tp[:, c * P : (c + 1) * P],
                                knat[:, dblk * P : (dblk + 1) * P],
                                ident,
                            )
                        nc.scalar.copy(
                            kt_r[:, g * 4 : g * 4 + 4, j * P : (j + 1) * P],
                            tp.rearrange("p (b k) -> p b k", b=4),
                        )

            with (
                tc.tile_pool(name="qnat", bufs=2) as qnatp,
                tc.tile_pool(name="qt", bufs=2) as qtp,
                tc.tile_pool(name="sw", bufs=2) as swp,
                tc.tile_pool(name="wt", bufs=1) as wtp,
                tc.tile_pool(name="cx", bufs=2) as cxp,
                tc.tile_pool(name="stat", bufs=2) as statp,
                tc.tile_pool(name="mf", bufs=2) as mfp,
            ):
                qt_tiles = {}
                sw_tiles = {}
                stat_tiles = {}

                def prefetch_q(i):
                    qnat = qnatp.tile([P, D], F32, tag="qnat")
                    nc.sync.dma_start(out=qnat, in_=qd[i * P : (i + 1) * P, :])
                    qt = qtp.tile([P, D], MMDT, tag="qt")
                    for g in range(ND // 4):
                        tp = psum_t.tile([P, 512], F32, tag="tp")
                        for c in range(4):
                            dblk = g * 4 + c
                            nc.tensor.transpose(
                                tp[:, c * P : (c + 1) * P],
                                qnat[:, dblk * P : (dblk + 1) * P],
                                ident,
                            )
                        # fold the 1/sqrt(D) score scaling into Q here:
                        # exact for power-of-two D (sqrt(1024)=32)
                        nc.scalar.mul(qt[:, g * 512 : (g + 1) * 512], tp, inv_temp)
                    qt_tiles[i] = qt

                def mm1_softmax(i):
                    qt = qt_tiles[i]
                    s_ps = psum_s.tile([P, KL], F32, tag="s")
                    for dblk in range(ND):
                        lq = _mm_cast(qt[:, dblk * P : (dblk + 1) * P])
                        for kc in range(NKC):
                            nc.tensor.matmul(
                                s_ps[:, kc * 512 : (kc + 1) * 512],
                                lhsT=lq,
                                rhs=_mm_cast(
                                    kt[:, dblk * KL + kc * 512 : dblk * KL + (kc + 1) * 512]
                                ),
                                start=(dblk == 0),
                                stop=(dblk == ND - 1),
                            )
                    sw = swp.tile([P, KL], F32, tag="sw")
                    if with_mask2d:
                        mf = mfp.tile([P, KL], F32, tag="mf")
                        nc.sync.dma_start(out=mf, in_=md[i * P : (i + 1) * P, :])
                        in1 = mf
                    else:
                        in1 = bias_b
                    # S_sbuf = S_psum + mask  (evacuates PSUM, applies mask)
                    nc.vector.tensor_add(sw, s_ps, in1)
                    negmax = statp.tile([P, 1], F32, tag="negmax")
                    nc.vector.reduce_max(
                        negmax, sw, axis=mybir.AxisListType.X, negate=True
                    )
                    rowsum = statp.tile([P, 1], F32, tag="rowsum")
                    nc.scalar.activation(
                        sw, sw, AF.Exp, bias=negmax, scale=1.0, accum_out=rowsum
                    )
                    recip = statp.tile([P, 1], F32, tag="recip")
                    nc.vector.reciprocal(recip, rowsum)
                    nc.vector.tensor_scalar_mul(sw, sw, recip)
                    nc.sync.dma_start(out=wd[i * P : (i + 1) * P, :], in_=sw)
                    sw_tiles[i] = sw

                def wt_mm2_ctx(i):
                    sw = sw_tiles.pop(i)
                    wt = wtp.tile([P, KL], MMDT, tag="wt")
                    for g in range(NKB // 4):
                        tp = psum_t.tile([P, 512], F32, tag="tp")
                        for c in range(4):
                            kblk = g * 4 + c
                            nc.tensor.transpose(
                                tp[:, c * P : (c + 1) * P],
                                sw[:, kblk * P : (kblk + 1) * P],
                                ident,
                            )
                        nc.scalar.copy(wt[:, g * 512 : (g + 1) * 512], tp)
                    c_ps = psum_c.tile([P, D], F32, tag="c")
                    for kblk in range(NKB):
                        lw = _mm_cast(wt[:, kblk * P : (kblk + 1) * P])
                        for dc in range(NDC):
                            nc.tensor.matmul(
                                c_ps[:, dc * 512 : (dc + 1) * 512],
                                lhsT=lw,
                                rhs=_mm_cast(
                                    vsb[:, kblk * D + dc * 512 : kblk * D + (dc + 1) * 512]
                                ),
                                start=(kblk == 0),
                                stop=(kblk == NKB - 1),
                            )
                    cx = cxp.tile([P, D], F32, tag="cx")
                    for dc in range(NDC):
                        nc.scalar.copy(
                            cx[:, dc * 512 : (dc + 1) * 512],
                            c_ps[:, dc * 512 : (dc + 1) * 512],
                        )
                    nc.sync.dma_start(out=ctxd[i * P : (i + 1) * P, :], in_=cx)

                prefetch_q(0)
                for i in range(NQ):
                    if i + 1 < NQ:
                        prefetch_q(i + 1)
                    mm1_softmax(i)
                    if i >= 1:
                        wt_mm2_ctx(i - 1)
                wt_mm2_ctx(NQ - 1)

    nc.compile()
    return nc


_CACHE = {}


def _get_program(QL, KL, D, with_mask2d):
    key = (QL, KL, D, with_mask2d)
    if key not in _CACHE:
        _CACHE[key] = build_attention(QL, KL, D, with_mask2d)
    return _CACHE[key]


def kernel(Q, K, V, attn_mask, key_pad_mask):
    global LAST_RESULTS
    Q = np.ascontiguousarray(np.asarray(Q, dtype=np.float32))
    K = np.ascontiguousarray(np.asarray(K, dtype=np.float32))
    V = np.ascontiguousarray(np.asarray(V, dtype=np.float32))
    attn_mask = np.asarray(attn_mask, dtype=np.float32)
    pad = np.asarray(key_pad_mask).astype(bool)

    B, QL, D = Q.shape
    KL = K.shape[1]
    assert B == N_CORES, f"expected batch {N_CORES}, got {B}"

    with_mask2d = bool(np.any(attn_mask))
    nc = _get_program(QL, KL, D, with_mask2d)

    bias = np.where(pad, np.float32(NEG_BIG), np.float32(0.0)).astype(np.float32)
    in_maps = []
    for b in range(B):
        m = {"q": Q[b], "k": K[b], "v": V[b]}
        if with_mask2d:
            m["mask2d"] = np.ascontiguousarray(
                attn_mask + bias[b][None, :], dtype=np.float32
            )
        else:
            m["bias"] = np.ascontiguousarray(bias[b][None, :])
        in_maps.append(m)

    res = run_bass_kernel_spmd(nc, in_maps, core_ids=list(range(N_CORES)))
    LAST_RESULTS = res

    ctx = np.stack([res.results[b]["ctx"] for b in range(B)])
    w = np.stack([res.results[b]["w"] for b in range(B)])
    return ctx, w


# revision 13
# speedup vs baseline: 1.0004x; 1.0004x over previous
"""Trainium2 Bass kernel for batched dot-product attention.

Problem: B=8, QL=KL=2048, D=1024 fp32.
  scores = Q @ K^T / sqrt(D) + attn_mask;  scores[pad] = -1e12
  W = softmax(scores, axis=-1);  ctx = W @ V
Returns (ctx, W) like the reference.

Sharding: pure data-parallel — one batch per NeuronCore, 8 cores.

Per-core kernel (SPMD program, same for all cores):
  - K, V loaded resident in SBUF; K transposed to [d, k] layout via PE
    transposes (fp32 has no DMA-transpose path).
  - Loop over 16 q-tiles of 128 rows:
      MM1  : S = Qt^T @ Kt  (float32r matmuls, N=512, accumulate over d)
      TTR  : S_sbuf = S_psum + bias_row (pad mask as -1e12 additive bias),
             fused row-max (DVE tensor_tensor_reduce)
      EXP  : E = exp(S - rowmax) fused row-sum (ACT activation accum_out)
      NORM : W = E * (1/rowsum)  (DVE tensor_scalar, in place)
      Wt   : PE-transpose W 128x128 blocks for MM2
      MM2  : ctx = Wt^T @ V    (float32r, accumulate over k)
  - Software-pipelined: MM2/transposes of tile i-1 are emitted after
    MM1/softmax of tile i so the PE never waits on the softmax chain.
"""

import os

import numpy as np

import concourse.bacc as bacc
import concourse.bass as bass
import concourse.mybir as mybir
import concourse.tile as tile
from concourse.bass_utils import run_bass_kernel_spmd
from concourse.masks import make_identity

F32 = mybir.dt.float32
F32R = mybir.dt.float32r
AF = mybir.ActivationFunctionType
ALU = mybir.AluOpType

P = 128
NEG_BIG = -1.0e12

N_CORES = 8
FULL_B, FULL_QL, FULL_KL, FULL_D = 8, 2048, 2048, 1024

# set to False to use plain fp32 matmuls (4x slower, exact) if fp32r
# accuracy turns out to be insufficient
USE_F32R = os.environ.get("ATTN_NO_F32R", "") == ""

LAST_RESULTS = None  # test harness introspection


MMDT = F32R if USE_F32R else F32


def _mm_cast(ap):
    return ap


def build_attention(QL, KL, D, with_mask2d):
    """Build the per-core SPMD Bass program.

    with_mask2d=False: inputs q,k,v,bias ([1,KL] additive row bias).
    with_mask2d=True : inputs q,k,v,mask2d ([QL,KL] additive mask =
                       attn_mask + pad bias, prepared on host).
    """
    inv_temp = float(1.0 / np.float32(np.sqrt(np.float32(D))))
    NQ = QL // P          # q tiles
    NKB = KL // P         # k blocks of 128
    ND = D // P           # d blocks of 128
    NKC = KL // 512       # k chunks of 512 (MM1 N dim)
    NDC = D // 512        # d chunks of 512 (MM2 N dim)
    assert QL % P == 0 and KL % 512 == 0 and D % 512 == 0

    nc = bacc.Bacc("TRN2", target_bir_lowering=False)

    qd = nc.declare_dram_parameter("q", [QL, D], F32, isOutput=False)
    kd = nc.declare_dram_parameter("k", [KL, D], F32, isOutput=False)
    vd = nc.declare_dram_parameter("v", [KL, D], F32, isOutput=False)
    if with_mask2d:
        md = nc.declare_dram_parameter("mask2d", [QL, KL], F32, isOutput=False)
    else:
        bd = nc.declare_dram_parameter("bias", [1, KL], F32, isOutput=False)
    ctxd = nc.declare_dram_parameter("ctx", [QL, D], F32, isOutput=True)
    wd = nc.declare_dram_parameter("w", [QL, KL], F32, isOutput=True)

    with tile.TileContext(nc) as tc:
        with (
            tc.tile_pool(name="const", bufs=1) as constp,
            tc.tile_pool(name="kv", bufs=1) as kvp,
            tc.tile_pool(name="psum_s", bufs=1, space="PSUM") as psum_s,
            tc.tile_pool(name="psum_c", bufs=1, space="PSUM") as psum_c,
            tc.tile_pool(name="psum_t", bufs=2, space="PSUM") as psum_t,
        ):
            ident = constp.tile([P, P], F32)
            make_identity(nc, ident)

            if not with_mask2d:
                bias_b = constp.tile([P, KL], F32)

            # K^T resident: kt[:, dblk*KL + kcol], partition = d within block
            kt = kvp.tile([P, ND * KL], MMDT)
            # V resident (natural): vsb[:, kblk*D + dcol], partition = k in blk
            vsb = kvp.tile([P, NKB * D], MMDT)
            kt_r = kt.rearrange("p (b k) -> p b k", b=ND)

            with (
                tc.tile_pool(name="qnat", bufs=2) as qnatp,
                tc.tile_pool(name="qt", bufs=2) as qtp,
                tc.tile_pool(name="sw", bufs=3) as swp,
                tc.tile_pool(name="wt", bufs=1) as wtp,
                tc.tile_pool(name="cx", bufs=2) as cxp,
                tc.tile_pool(name="stat", bufs=2) as statp,
                tc.tile_pool(name="mf", bufs=1) as mfp,
            ):
                qt_tiles = {}
                sw_tiles = {}

                def prefetch_q(i):
                    qnat = qnatp.tile([P, D], F32, tag="qnat")
                    nc.sync.dma_start(out=qnat, in_=qd[i * P : (i + 1) * P, :])
                    qt = qtp.tile([P, D], MMDT, tag="qt")
                    for g in range(ND // 4):
                        tp = psum_t.tile([P, 512], F32, tag="tp")
                        for c in range(4):
                            dblk = g * 4 + c
                            nc.tensor.transpose(
                                tp[:, c * P : (c + 1) * P],
                                qnat[:, dblk * P : (dblk + 1) * P],
                                ident,
                            )
                        # fold the 1/sqrt(D) score scaling into Q here:
                        # exact for power-of-two D (sqrt(1024)=32)
                        nc.scalar.mul(qt[:, g * 512 : (g + 1) * 512], tp, inv_temp)
                    qt_tiles[i] = qt

                def mm1_chunk(i, s_ps, kc):
                    # one 512-wide k-chunk of S for q-tile i, accumulated
                    # over all 8 d-blocks (needs only k-tiles 4kc..4kc+3)
                    qt = qt_tiles[i]
                    for dblk in range(ND):
                        nc.tensor.matmul(
                            s_ps[:, kc * 512 : (kc + 1) * 512],
                            lhsT=qt[:, dblk * P : (dblk + 1) * P],
                            rhs=kt[
                                :, dblk * KL + kc * 512 : dblk * KL + (kc + 1) * 512
                            ],
                            start=(dblk == 0),
                            stop=(dblk == ND - 1),
                        )

                def softmax(i, s_ps):
                    sw = swp.tile([P, KL], F32, tag="sw")
                    if with_mask2d:
                        mf = mfp.tile([P, KL], F32, tag="mf")
                        nc.sync.dma_start(out=mf, in_=md[i * P : (i + 1) * P, :])
                        in1 = mf
                    else:
                        in1 = bias_b
                    # S_sbuf = S_psum + mask  (evacuates PSUM, applies mask)
                    nc.vector.tensor_add(sw, s_ps, in1)
                    negmax = statp.tile([P, 1], F32, tag="negmax")
                    nc.vector.reduce_max(
                        negmax, sw, axis=mybir.AxisListType.X, negate=True
                    )
                    rowsum = statp.tile([P, 1], F32, tag="rowsum")
                    nc.scalar.activation(
                        sw, sw, AF.Exp, bias=negmax, scale=1.0, accum_out=rowsum
                    )
                    recip = statp.tile([P, 1], F32, tag="recip")
                    nc.vector.reciprocal(recip, rowsum)
                    nc.vector.tensor_scalar_mul(sw, sw, recip)
                    nc.sync.dma_start(out=wd[i * P : (i + 1) * P, :], in_=sw)
                    sw_tiles[i] = sw

                def wt_mm2_ctx(i):
                    sw = sw_tiles.pop(i)
                    wt = wtp.tile([P, KL], MMDT, tag="wt")
                    for g in range(NKB // 4):
                        tp = psum_t.tile([P, 512], F32, tag="tp")
                        for c in range(4):
                            kblk = g * 4 + c
                            nc.tensor.transpose(
                                tp[:, c * P : (c + 1) * P],
                                sw[:, kblk * P : (kblk + 1) * P],
                                ident,
                            )
                        nc.scalar.copy(wt[:, g * 512 : (g + 1) * 512], tp)
                    c_ps = psum_c.tile([P, D], F32, tag="c")
                    for kblk in range(NKB):
                        lw = _mm_cast(wt[:, kblk * P : (kblk + 1) * P])
                        for dc in range(NDC):
                            nc.tensor.matmul(
                                c_ps[:, dc * 512 : (dc + 1) * 512],
                                lhsT=lw,
                                rhs=_mm_cast(
                                    vsb[:, kblk * D + dc * 512 : kblk * D + (dc + 1) * 512]
                                ),
                                start=(kblk == 0),
                                stop=(kblk == NKB - 1),
                            )
                    cx = cxp.tile([P, D], F32, tag="cx")
                    for dc in range(NDC):
                        nc.scalar.copy(
                            cx[:, dc * 512 : (dc + 1) * 512],
                            c_ps[:, dc * 512 : (dc + 1) * 512],
                        )
                    nc.sync.dma_start(out=ctxd[i * P : (i + 1) * P, :], in_=cx)

                # ---- prologue: Q(0), then K in groups of 4 tiles with
                # tile-0 MM1 chunks interleaved, then V ----
                prefetch_q(0)
                s_ps0 = psum_s.tile([P, KL], F32, tag="s")
                with tc.tile_pool(name="knat", bufs=2) as knatp:
                    if not with_mask2d:
                        # replicate the [1, KL] pad-bias row to all partitions
                        nc.sync.dma_start(
                            out=bias_b, in_=bd[0].partition_broadcast(P)
                        )
                    for g in range(NKC):
                        for j in range(4 * g, 4 * g + 4):
                            knat = knatp.tile([P, D], F32, tag="knat")
                            nc.sync.dma_start(out=knat, in_=kd[j * P : (j + 1) * P, :])
                            for tg in range(ND // 4):
                                tp = psum_t.tile([P, 512], F32, tag="tp")
                                for c in range(4):
                                    dblk = tg * 4 + c
                                    nc.tensor.transpose(
                                        tp[:, c * P : (c + 1) * P],
                                        knat[:, dblk * P : (dblk + 1) * P],
                                        ident,
                                    )
                                nc.scalar.copy(
                                    kt_r[:, tg * 4 : tg * 4 + 4, j * P : (j + 1) * P],
                                    tp.rearrange("p (b k) -> p b k", b=4),
                                )
                        mm1_chunk(0, s_ps0, g)
                for j in range(NKB):
                    nc.sync.dma_start(
                        out=vsb[:, j * D : (j + 1) * D],
                        in_=vd[j * P : (j + 1) * P, :].bitcast(MMDT),
                    )
                prefetch_q(1)
                softmax(0, s_ps0)

                for i in range(1, NQ):
                    if i + 1 < NQ:
                        prefetch_q(i + 1)
                    s_ps = psum_s.tile([P, KL], F32, tag="s")
                    for kc in range(NKC):
                        mm1_chunk(i, s_ps, kc)
                    softmax(i, s_ps)
                    if i >= 2:
                        wt_mm2_ctx(i - 2)
                wt_mm2_ctx(NQ - 2)
                wt_mm2_ctx(NQ - 1)

    nc.compile()
    return nc


_CACHE = {}


def _get_program(QL, KL, D, with_mask2d):
    key = (QL, KL, D, with_mask2d)
    if key not in _CACHE:
        _CACHE[key] = build_attention(QL, KL, D, with_mask2d)
    return _CACHE[key]


def kernel(Q, K, V, attn_mask, key_pad_mask):
    global LAST_RESULTS
    Q = np.ascontiguousarray(np.asarray(Q, dtype=np.float32))
    K = np.ascontiguousarray(np.asarray(K, dtype=np.float32))
    V = np.ascontiguousarray(np.asarray(V, dtype=np.float32))
    attn_mask = np.asarray(attn_mask, dtype=np.float32)
    pad = np.asarray(key_pad_mask).astype(bool)

    B, QL, D = Q.shape
    KL = K.shape[1]
    assert B == N_CORES, f"expected batch {N_CORES}, got {B}"

    with_mask2d = bool(np.any(attn_mask))
    nc = _get_program(QL, KL, D, with_mask2d)

    bias = np.where(pad, np.float32(NEG_BIG), np.float32(0.0)).astype(np.float32)
    in_maps = []
    for b in range(B):
        m = {"q": Q[b], "k": K[b], "v": V[b]}
        if with_mask2d:
            m["mask2d"] = np.ascontiguousarray(
                attn_mask + bias[b][None, :], dtype=np.float32
            )
        else:
            m["bias"] = np.ascontiguousarray(bias[b][None, :])
        in_maps.append(m)

    res = run_bass_kernel_spmd(nc, in_maps, core_ids=list(range(N_CORES)))
    LAST_RESULTS = res

    ctx = np.stack([res.results[b]["ctx"] for b in range(B)])
    w = np.stack([res.results[b]["w"] for b in range(B)])
    return ctx, w


# revision 14
# speedup vs baseline: 1.0010x; 1.0006x over previous
"""Trainium2 Bass kernel for batched dot-product attention.

Problem: B=8, QL=KL=2048, D=1024 fp32.
  scores = Q @ K^T / sqrt(D) + attn_mask;  scores[pad] = -1e12
  W = softmax(scores, axis=-1);  ctx = W @ V
Returns (ctx, W) like the reference.

Sharding: pure data-parallel — one batch per NeuronCore, 8 cores.

Per-core kernel (SPMD program, same for all cores):
  - K, V loaded resident in SBUF; K transposed to [d, k] layout via PE
    transposes (fp32 has no DMA-transpose path).
  - Loop over 16 q-tiles of 128 rows:
      MM1  : S = Qt^T @ Kt  (float32r matmuls, N=512, accumulate over d)
      TTR  : S_sbuf = S_psum + bias_row (pad mask as -1e12 additive bias),
             fused row-max (DVE tensor_tensor_reduce)
      EXP  : E = exp(S - rowmax) fused row-sum (ACT activation accum_out)
      NORM : W = E * (1/rowsum)  (DVE tensor_scalar, in place)
      Wt   : PE-transpose W 128x128 blocks for MM2
      MM2  : ctx = Wt^T @ V    (float32r, accumulate over k)
  - Software-pipelined: MM2/transposes of tile i-1 are emitted after
    MM1/softmax of tile i so the PE never waits on the softmax chain.
"""

import os

import numpy as np

import concourse.bacc as bacc
import concourse.bass as bass
import concourse.mybir as mybir
import concourse.tile as tile
from concourse.bass_utils import run_bass_kernel_spmd

F32 = mybir.dt.float32
F32R = mybir.dt.float32r
AF = mybir.ActivationFunctionType
ALU = mybir.AluOpType

P = 128
NEG_BIG = -1.0e12

N_CORES = 8
FULL_B, FULL_QL, FULL_KL, FULL_D = 8, 2048, 2048, 1024

# set to False to use plain fp32 matmuls (4x slower, exact) if fp32r
# accuracy turns out to be insufficient
USE_F32R = os.environ.get("ATTN_NO_F32R", "") == ""

LAST_RESULTS = None  # test harness introspection


MMDT = F32R if USE_F32R else F32


def _mm_cast(ap):
    return ap


def build_attention(QL, KL, D, with_mask2d):
    """Build the per-core SPMD Bass program.

    with_mask2d=False: inputs q,k,v,bias ([1,KL] additive row bias).
    with_mask2d=True : inputs q,k,v,mask2d ([QL,KL] additive mask =
                       attn_mask + pad bias, prepared on host).
    """
    inv_temp = float(1.0 / np.float32(np.sqrt(np.float32(D))))
    NQ = QL // P          # q tiles
    NKB = KL // P         # k blocks of 128
    ND = D // P           # d blocks of 128
    NKC = KL // 512       # k chunks of 512 (MM1 N dim)
    NDC = D // 512        # d chunks of 512 (MM2 N dim)
    assert QL % P == 0 and KL % 512 == 0 and D % 512 == 0

    nc = bacc.Bacc("TRN2", target_bir_lowering=False)

    qd = nc.declare_dram_parameter("q", [QL, D], F32, isOutput=False)
    identd = nc.declare_dram_parameter("ident", [P, P], F32, isOutput=False)
    kd = nc.declare_dram_parameter("k", [KL, D], F32, isOutput=False)
    vd = nc.declare_dram_parameter("v", [KL, D], F32, isOutput=False)
    if with_mask2d:
        md = nc.declare_dram_parameter("mask2d", [QL, KL], F32, isOutput=False)
    else:
        bd = nc.declare_dram_parameter("bias", [1, KL], F32, isOutput=False)
    ctxd = nc.declare_dram_parameter("ctx", [QL, D], F32, isOutput=True)
    wd = nc.declare_dram_parameter("w", [QL, KL], F32, isOutput=True)

    with tile.TileContext(nc) as tc:
        with (
            tc.tile_pool(name="const", bufs=1) as constp,
            tc.tile_pool(name="kv", bufs=1) as kvp,
            tc.tile_pool(name="psum_s", bufs=1, space="PSUM") as psum_s,
            tc.tile_pool(name="psum_c", bufs=1, space="PSUM") as psum_c,
            tc.tile_pool(name="psum_t", bufs=2, space="PSUM") as psum_t,
        ):
            ident = constp.tile([P, P], F32)
            nc.sync.dma_start(out=ident, in_=identd[:, :])

            if not with_mask2d:
                bias_b = constp.tile([P, KL], F32)

            # K^T resident: kt[:, dblk*KL + kcol], partition = d within block
            kt = kvp.tile([P, ND * KL], MMDT)
            # V resident (natural): vsb[:, kblk*D + dcol], partition = k in blk
            vsb = kvp.tile([P, NKB * D], MMDT)
            kt_r = kt.rearrange("p (b k) -> p b k", b=ND)

            with (
                tc.tile_pool(name="qnat", bufs=2) as qnatp,
                tc.tile_pool(name="qt", bufs=2) as qtp,
                tc.tile_pool(name="sw", bufs=4) as swp,
                tc.tile_pool(name="wt", bufs=1) as wtp,
                tc.tile_pool(name="cx", bufs=1) as cxp,
                tc.tile_pool(name="stat", bufs=2) as statp,
                tc.tile_pool(name="mf", bufs=1) as mfp,
            ):
                qt_tiles = {}
                sw_tiles = {}

                def prefetch_q(i):
                    qnat = qnatp.tile([P, D], F32, tag="qnat")
                    nc.sync.dma_start(out=qnat, in_=qd[i * P : (i + 1) * P, :])
                    qt = qtp.tile([P, D], MMDT, tag="qt")
                    for g in range(ND // 4):
                        tp = psum_t.tile([P, 512], F32, tag="tp")
                        for c in range(4):
                            dblk = g * 4 + c
                            nc.tensor.transpose(
                                tp[:, c * P : (c + 1) * P],
                                qnat[:, dblk * P : (dblk + 1) * P],
                                ident,
                            )
                        # fold the 1/sqrt(D) score scaling into Q here:
                        # exact for power-of-two D (sqrt(1024)=32)
                        nc.scalar.mul(qt[:, g * 512 : (g + 1) * 512], tp, inv_temp)
                    qt_tiles[i] = qt

                def mm1_chunk(i, s_ps, kc):
                    # one 512-wide k-chunk of S for q-tile i, accumulated
                    # over all 8 d-blocks (needs only k-tiles 4kc..4kc+3)
                    qt = qt_tiles[i]
                    for dblk in range(ND):
                        nc.tensor.matmul(
                            s_ps[:, kc * 512 : (kc + 1) * 512],
                            lhsT=qt[:, dblk * P : (dblk + 1) * P],
                            rhs=kt[
                                :, dblk * KL + kc * 512 : dblk * KL + (kc + 1) * 512
                            ],
                            start=(dblk == 0),
                            stop=(dblk == ND - 1),
                        )

                def softmax(i, s_ps):
                    sw = swp.tile([P, KL], F32, tag="sw")
                    if with_mask2d:
                        mf = mfp.tile([P, KL], F32, tag="mf")
                        nc.sync.dma_start(out=mf, in_=md[i * P : (i + 1) * P, :])
                        in1 = mf
                    else:
                        in1 = bias_b
                    # S_sbuf = S_psum + mask  (evacuates PSUM, applies mask)
                    nc.vector.tensor_add(sw, s_ps, in1)
                    negmax = statp.tile([P, 1], F32, tag="negmax")
                    nc.vector.reduce_max(
                        negmax, sw, axis=mybir.AxisListType.X, negate=True
                    )
                    rowsum = statp.tile([P, 1], F32, tag="rowsum")
                    nc.scalar.activation(
                        sw, sw, AF.Exp, bias=negmax, scale=1.0, accum_out=rowsum
                    )
                    recip = statp.tile([P, 1], F32, tag="recip")
                    nc.vector.reciprocal(recip, rowsum)
                    nc.vector.tensor_scalar_mul(sw, sw, recip)
                    nc.sync.dma_start(out=wd[i * P : (i + 1) * P, :], in_=sw)
                    sw_tiles[i] = sw

                def wt_mm2_ctx(i):
                    sw = sw_tiles.pop(i)
                    wt = wtp.tile([P, KL], MMDT, tag="wt")
                    for g in range(NKB // 4):
                        tp = psum_t.tile([P, 512], F32, tag="tp")
                        for c in range(4):
                            kblk = g * 4 + c
                            nc.tensor.transpose(
                                tp[:, c * P : (c + 1) * P],
                                sw[:, kblk * P : (kblk + 1) * P],
                                ident,
                            )
                        nc.scalar.copy(wt[:, g * 512 : (g + 1) * 512], tp)
                    c_ps = psum_c.tile([P, D], F32, tag="c")
                    for kblk in range(NKB):
                        lw = _mm_cast(wt[:, kblk * P : (kblk + 1) * P])
                        for dc in range(NDC):
                            nc.tensor.matmul(
                                c_ps[:, dc * 512 : (dc + 1) * 512],
                                lhsT=lw,
                                rhs=_mm_cast(
                                    vsb[:, kblk * D + dc * 512 : kblk * D + (dc + 1) * 512]
                                ),
                                start=(kblk == 0),
                                stop=(kblk == NKB - 1),
                            )
                    cx = cxp.tile([P, D], F32, tag="cx")
                    for dc in range(NDC):
                        nc.scalar.copy(
                            cx[:, dc * 512 : (dc + 1) * 512],
                            c_ps[:, dc * 512 : (dc + 1) * 512],
                        )
                    nc.sync.dma_start(out=ctxd[i * P : (i + 1) * P, :], in_=cx)

                # ---- prologue: Q(0), then K in groups of 4 tiles with
                # tile-0 MM1 chunks interleaved, then V ----
                prefetch_q(0)
                s_ps0 = psum_s.tile([P, KL], F32, tag="s")
                with tc.tile_pool(name="knat", bufs=2) as knatp:
                    if not with_mask2d:
                        # replicate the [1, KL] pad-bias row to all partitions
                        nc.sync.dma_start(
                            out=bias_b, in_=bd[0].partition_broadcast(P)
                        )
                    for g in range(NKC):
                        for j in range(4 * g, 4 * g + 4):
                            knat = knatp.tile([P, D], F32, tag="knat")
                            nc.sync.dma_start(out=knat, in_=kd[j * P : (j + 1) * P, :])
                            for tg in range(ND // 4):
                                tp = psum_t.tile([P, 512], F32, tag="tp")
                                for c in range(4):
                                    dblk = tg * 4 + c
                                    nc.tensor.transpose(
                                        tp[:, c * P : (c + 1) * P],
                                        knat[:, dblk * P : (dblk + 1) * P],
                                        ident,
                                    )
                                nc.scalar.copy(
                                    kt_r[:, tg * 4 : tg * 4 + 4, j * P : (j + 1) * P],
                                    tp.rearrange("p (b k) -> p b k", b=4),
                                )
                        mm1_chunk(0, s_ps0, g)
                for j in range(NKB):
                    nc.sync.dma_start(
                        out=vsb[:, j * D : (j + 1) * D],
                        in_=vd[j * P : (j + 1) * P, :].bitcast(MMDT),
                    )
                prefetch_q(1)
                softmax(0, s_ps0)

                DEPTH = 3
                for i in range(1, NQ):
                    if i + 1 < NQ:
                        prefetch_q(i + 1)
                    s_ps = psum_s.tile([P, KL], F32, tag="s")
                    for kc in range(NKC):
                        mm1_chunk(i, s_ps, kc)
                    softmax(i, s_ps)
                    if i >= DEPTH:
                        wt_mm2_ctx(i - DEPTH)
                for i in range(max(NQ - DEPTH, 0), NQ):
                    wt_mm2_ctx(i)

    nc.compile()
    return nc


_CACHE = {}


def _get_program(QL, KL, D, with_mask2d):
    key = (QL, KL, D, with_mask2d)
    if key not in _CACHE:
        _CACHE[key] = build_attention(QL, KL, D, with_mask2d)
    return _CACHE[key]


def kernel(Q, K, V, attn_mask, key_pad_mask):
    global LAST_RESULTS
    Q = np.ascontiguousarray(np.asarray(Q, dtype=np.float32))
    K = np.ascontiguousarray(np.asarray(K, dtype=np.float32))
    V = np.ascontiguousarray(np.asarray(V, dtype=np.float32))
    attn_mask = np.asarray(attn_mask, dtype=np.float32)
    pad = np.asarray(key_pad_mask).astype(bool)

    B, QL, D = Q.shape
    KL = K.shape[1]
    assert B == N_CORES, f"expected batch {N_CORES}, got {B}"

    with_mask2d = bool(np.any(attn_mask))
    nc = _get_program(QL, KL, D, with_mask2d)

    bias = np.where(pad, np.float32(NEG_BIG), np.float32(0.0)).astype(np.float32)
    ident_np = np.eye(P, dtype=np.float32)
    in_maps = []
    for b in range(B):
        m = {"q": Q[b], "k": K[b], "v": V[b], "ident": ident_np}
        if with_mask2d:
            m["mask2d"] = np.ascontiguousarray(
                attn_mask + bias[b][None, :], dtype=np.float32
            )
        else:
            m["bias"] = np.ascontiguousarray(bias[b][None, :])
        in_maps.append(m)

    res = run_bass_kernel_spmd(nc, in_maps, core_ids=list(range(N_CORES)))
    LAST_RESULTS = res

    ctx = np.stack([res.results[b]["ctx"] for b in range(B)])
    w = np.stack([res.results[b]["w"] for b in range(B)])
    return ctx, w


# revision 15
# speedup vs baseline: 1.0181x; 1.0171x over previous
"""Trainium2 Bass kernel for batched dot-product attention.

Problem: B=8, QL=KL=2048, D=1024 fp32.
  scores = Q @ K^T / sqrt(D) + attn_mask;  scores[pad] = -1e12
  W = softmax(scores, axis=-1);  ctx = W @ V
Returns (ctx, W) like the reference.

Sharding: pure data-parallel — one batch per NeuronCore, 8 cores.

Per-core kernel (SPMD program, same for all cores):
  - K, V loaded resident in SBUF; K transposed to [d, k] layout via PE
    transposes (fp32 has no DMA-transpose path).
  - Loop over 16 q-tiles of 128 rows:
      MM1  : S = Qt^T @ Kt  (float32r matmuls, N=512, accumulate over d)
      TTR  : S_sbuf = S_psum + bias_row (pad mask as -1e12 additive bias),
             fused row-max (DVE tensor_tensor_reduce)
      EXP  : E = exp(S - rowmax) fused row-sum (ACT activation accum_out)
      NORM : W = E * (1/rowsum)  (DVE tensor_scalar, in place)
      Wt   : PE-transpose W 128x128 blocks for MM2
      MM2  : ctx = Wt^T @ V    (float32r, accumulate over k)
  - Software-pipelined: MM2/transposes of tile i-1 are emitted after
    MM1/softmax of tile i so the PE never waits on the softmax chain.
"""

import os

import numpy as np

import concourse.bacc as bacc
import concourse.bass as bass
import concourse.mybir as mybir
import concourse.tile as tile
from concourse.bass_utils import run_bass_kernel_spmd

F32 = mybir.dt.float32
F32R = mybir.dt.float32r
AF = mybir.ActivationFunctionType
ALU = mybir.AluOpType

P = 128
NEG_BIG = -1.0e12

N_CORES = 8
FULL_B, FULL_QL, FULL_KL, FULL_D = 8, 2048, 2048, 1024

# set to False to use plain fp32 matmuls (4x slower, exact) if fp32r
# accuracy turns out to be insufficient
USE_F32R = os.environ.get("ATTN_NO_F32R", "") == ""

LAST_RESULTS = None  # test harness introspection


MMDT = F32R if USE_F32R else F32


def _mm_cast(ap):
    return ap


def build_attention(QL, KL, D, with_mask2d):
    """Build the per-core SPMD Bass program.

    with_mask2d=False: inputs q,k,v,bias ([1,KL] additive row bias).
    with_mask2d=True : inputs q,k,v,mask2d ([QL,KL] additive mask =
                       attn_mask + pad bias, prepared on host).
    """
    inv_temp = float(1.0 / np.float32(np.sqrt(np.float32(D))))
    NQ = QL // P          # q tiles
    NKB = KL // P         # k blocks of 128
    ND = D // P           # d blocks of 128
    NKC = KL // 512       # k chunks of 512 (MM1 N dim)
    NDC = D // 512        # d chunks of 512 (MM2 N dim)
    assert QL % P == 0 and KL % 512 == 0 and D % 512 == 0

    nc = bacc.Bacc("TRN2", target_bir_lowering=False)

    qd = nc.declare_dram_parameter("q", [QL, D], F32, isOutput=False)
    identd = nc.declare_dram_parameter("ident", [P, P], F32, isOutput=False)
    kd = nc.declare_dram_parameter("k", [KL, D], F32, isOutput=False)
    vd = nc.declare_dram_parameter("v", [KL, D], F32, isOutput=False)
    if with_mask2d:
        md = nc.declare_dram_parameter("mask2d", [QL, KL], F32, isOutput=False)
    else:
        bd = nc.declare_dram_parameter("bias", [1, KL], F32, isOutput=False)
    ctxd = nc.declare_dram_parameter("ctx", [QL, D], F32, isOutput=True)
    wd = nc.declare_dram_parameter("w", [QL, KL], F32, isOutput=True)

    with tile.TileContext(nc) as tc:
        with (
            tc.tile_pool(name="const", bufs=1) as constp,
            tc.tile_pool(name="kv", bufs=1) as kvp,
            tc.tile_pool(name="psum_s", bufs=1, space="PSUM") as psum_s,
            tc.tile_pool(name="psum_c", bufs=1, space="PSUM") as psum_c,
            tc.tile_pool(name="psum_t", bufs=2, space="PSUM") as psum_t,
        ):
            ident = constp.tile([P, P], F32)
            nc.sync.dma_start(out=ident, in_=identd[:, :])

            if not with_mask2d:
                bias_b = constp.tile([P, KL], F32)

            # K^T resident: kt[:, dblk*KL + kcol], partition = d within block
            kt = kvp.tile([P, ND * KL], MMDT)
            # V resident (natural): vsb[:, kblk*D + dcol], partition = k in blk
            vsb = kvp.tile([P, NKB * D], MMDT)
            kt_r = kt.rearrange("p (b k) -> p b k", b=ND)

            with (
                tc.tile_pool(name="qnat", bufs=2) as qnatp,
                tc.tile_pool(name="qt", bufs=2) as qtp,
                tc.tile_pool(name="sw", bufs=3) as swp,
                tc.tile_pool(name="wt", bufs=1) as wtp,
                tc.tile_pool(name="cx", bufs=1) as cxp,
                tc.tile_pool(name="stat", bufs=2) as statp,
                tc.tile_pool(name="mf", bufs=1) as mfp,
            ):
                qt_tiles = {}
                sw_tiles = {}

                def prefetch_q(i):
                    qnat = qnatp.tile([P, D], F32, tag="qnat")
                    nc.sync.dma_start(out=qnat, in_=qd[i * P : (i + 1) * P, :])
                    qt = qtp.tile([P, D], MMDT, tag="qt")
                    for g in range(ND // 4):
                        tp = psum_t.tile([P, 512], F32, tag="tp")
                        for c in range(4):
                            dblk = g * 4 + c
                            nc.tensor.transpose(
                                tp[:, c * P : (c + 1) * P],
                                qnat[:, dblk * P : (dblk + 1) * P],
                                ident,
                            )
                        # fold the 1/sqrt(D) score scaling into Q here:
                        # exact for power-of-two D (sqrt(1024)=32)
                        nc.scalar.mul(qt[:, g * 512 : (g + 1) * 512], tp, inv_temp)
                    qt_tiles[i] = qt

                def mm1_chunk(i, s_ps, kc):
                    # one 512-wide k-chunk of S for q-tile i, accumulated
                    # over all 8 d-blocks (needs only k-tiles 4kc..4kc+3)
                    qt = qt_tiles[i]
                    for dblk in range(ND):
                        nc.tensor.matmul(
                            s_ps[:, kc * 512 : (kc + 1) * 512],
                            lhsT=qt[:, dblk * P : (dblk + 1) * P],
                            rhs=kt[
                                :, dblk * KL + kc * 512 : dblk * KL + (kc + 1) * 512
                            ],
                            start=(dblk == 0),
                            stop=(dblk == ND - 1),
                        )

                def softmax(i, s_ps):
                    sw = swp.tile([P, KL], F32, tag="sw")
                    if with_mask2d:
                        mf = mfp.tile([P, KL], F32, tag="mf")
                        nc.sync.dma_start(out=mf, in_=md[i * P : (i + 1) * P, :])
                        in1 = mf
                    else:
                        in1 = bias_b
                    # S_sbuf = S_psum + mask  (evacuates PSUM, applies mask)
                    nc.vector.tensor_add(sw, s_ps, in1)
                    negmax = statp.tile([P, 1], F32, tag="negmax")
                    nc.vector.reduce_max(
                        negmax, sw, axis=mybir.AxisListType.X, negate=True
                    )
                    rowsum = statp.tile([P, 1], F32, tag="rowsum")
                    nc.scalar.activation(
                        sw, sw, AF.Exp, bias=negmax, scale=1.0, accum_out=rowsum
                    )
                    recip = statp.tile([P, 1], F32, tag="recip")
                    nc.vector.reciprocal(recip, rowsum)
                    nc.vector.tensor_scalar_mul(sw, sw, recip)
                    nc.sync.dma_start(out=wd[i * P : (i + 1) * P, :], in_=sw)
                    sw_tiles[i] = sw

                def wt_mm2_ctx(i):
                    sw = sw_tiles.pop(i)
                    wt = wtp.tile([P, KL], MMDT, tag="wt")
                    for g in range(NKB // 4):
                        tp = psum_t.tile([P, 512], F32, tag="tp")
                        for c in range(4):
                            kblk = g * 4 + c
                            nc.tensor.transpose(
                                tp[:, c * P : (c + 1) * P],
                                sw[:, kblk * P : (kblk + 1) * P],
                                ident,
                            )
                        nc.scalar.copy(wt[:, g * 512 : (g + 1) * 512], tp)
                    c_ps = psum_c.tile([P, D], F32, tag="c")
                    for kblk in range(NKB):
                        lw = _mm_cast(wt[:, kblk * P : (kblk + 1) * P])
                        for dc in range(NDC):
                            nc.tensor.matmul(
                                c_ps[:, dc * 512 : (dc + 1) * 512],
                                lhsT=lw,
                                rhs=_mm_cast(
                                    vsb[:, kblk * D + dc * 512 : kblk * D + (dc + 1) * 512]
                                ),
                                start=(kblk == 0),
                                stop=(kblk == NKB - 1),
                            )
                    cx = cxp.tile([P, D], F32, tag="cx")
                    for dc in range(NDC):
                        nc.scalar.copy(
                            cx[:, dc * 512 : (dc + 1) * 512],
                            c_ps[:, dc * 512 : (dc + 1) * 512],
                        )
                    nc.sync.dma_start(out=ctxd[i * P : (i + 1) * P, :], in_=cx)

                # ---- prologue: Q(0), then K in groups of 4 tiles with
                # tile-0 MM1 chunks interleaved, then V ----
                prefetch_q(0)
                s_ps0 = psum_s.tile([P, KL], F32, tag="s")
                with tc.tile_pool(name="knat", bufs=4) as knatp:
                    if not with_mask2d:
                        # replicate the [1, KL] pad-bias row to all partitions
                        nc.sync.dma_start(
                            out=bias_b, in_=bd[0].partition_broadcast(P)
                        )
                    for g in range(NKC):
                        for j in range(4 * g, 4 * g + 4):
                            knat = knatp.tile([P, D], F32, tag="knat")
                            nc.sync.dma_start(out=knat, in_=kd[j * P : (j + 1) * P, :])
                            for tg in range(ND // 4):
                                tp = psum_t.tile([P, 512], F32, tag="tp")
                                for c in range(4):
                                    dblk = tg * 4 + c
                                    nc.tensor.transpose(
                                        tp[:, c * P : (c + 1) * P],
                                        knat[:, dblk * P : (dblk + 1) * P],
                                        ident,
                                    )
                                nc.scalar.copy(
                                    kt_r[:, tg * 4 : tg * 4 + 4, j * P : (j + 1) * P],
                                    tp.rearrange("p (b k) -> p b k", b=4),
                                )
                        mm1_chunk(0, s_ps0, g)
                for j in range(NKB):
                    nc.sync.dma_start(
                        out=vsb[:, j * D : (j + 1) * D],
                        in_=vd[j * P : (j + 1) * P, :].bitcast(MMDT),
                    )
                prefetch_q(1)
                softmax(0, s_ps0)

                DEPTH = 2
                for i in range(1, NQ):
                    if i + 1 < NQ:
                        prefetch_q(i + 1)
                    s_ps = psum_s.tile([P, KL], F32, tag="s")
                    for kc in range(NKC):
                        mm1_chunk(i, s_ps, kc)
                    softmax(i, s_ps)
                    if i >= DEPTH:
                        wt_mm2_ctx(i - DEPTH)
                for i in range(max(NQ - DEPTH, 0), NQ):
                    wt_mm2_ctx(i)

    nc.compile()
    return nc


_CACHE = {}


def _get_program(QL, KL, D, with_mask2d):
    key = (QL, KL, D, with_mask2d)
    if key not in _CACHE:
        _CACHE[key] = build_attention(QL, KL, D, with_mask2d)
    return _CACHE[key]


def kernel(Q, K, V, attn_mask, key_pad_mask):
    global LAST_RESULTS
    Q = np.ascontiguousarray(np.asarray(Q, dtype=np.float32))
    K = np.ascontiguousarray(np.asarray(K, dtype=np.float32))
    V = np.ascontiguousarray(np.asarray(V, dtype=np.float32))
    attn_mask = np.asarray(attn_mask, dtype=np.float32)
    pad = np.asarray(key_pad_mask).astype(bool)

    B, QL, D = Q.shape
    KL = K.shape[1]
    assert B == N_CORES, f"expected batch {N_CORES}, got {B}"

    with_mask2d = bool(np.any(attn_mask))
    nc = _get_program(QL, KL, D, with_mask2d)

    bias = np.where(pad, np.float32(NEG_BIG), np.float32(0.0)).astype(np.float32)
    ident_np = np.eye(P, dtype=np.float32)
    in_maps = []
    for b in range(B):
        m = {"q": Q[b], "k": K[b], "v": V[b], "ident": ident_np}
        if with_mask2d:
            m["mask2d"] = np.ascontiguousarray(
                attn_mask + bias[b][None, :], dtype=np.float32
            )
        else:
            m["bias"] = np.ascontiguousarray(bias[b][None, :])
        in_maps.append(m)

    res = run_bass_kernel_spmd(nc, in_maps, core_ids=list(range(N_CORES)))
    LAST_RESULTS = res

    ctx = np.stack([res.results[b]["ctx"] for b in range(B)])
    w = np.stack([res.results[b]["w"] for b in range(B)])
    return ctx, w


# revision 16
# speedup vs baseline: 1.0650x; 1.0461x over previous
"""Trainium2 Bass kernel for batched dot-product attention.

Problem: B=8, QL=KL=2048, D=1024 fp32.
  scores = Q @ K^T / sqrt(D) + attn_mask;  scores[pad] = -1e12
  W = softmax(scores, axis=-1);  ctx = W @ V
Returns (ctx, W) like the reference.

Sharding: pure data-parallel — one batch per NeuronCore, 8 cores.

Per-core kernel (SPMD program, same for all cores):
  - K, V loaded resident in SBUF; K transposed to [d, k] layout via PE
    transposes (fp32 has no DMA-transpose path).
  - Loop over 16 q-tiles of 128 rows:
      MM1  : S = Qt^T @ Kt  (float32r matmuls, N=512, accumulate over d)
      TTR  : S_sbuf = S_psum + bias_row (pad mask as -1e12 additive bias),
             fused row-max (DVE tensor_tensor_reduce)
      EXP  : E = exp(S - rowmax) fused row-sum (ACT activation accum_out)
      NORM : W = E * (1/rowsum)  (DVE tensor_scalar, in place)
      Wt   : PE-transpose W 128x128 blocks for MM2
      MM2  : ctx = Wt^T @ V    (float32r, accumulate over k)
  - Software-pipelined: MM2/transposes of tile i-1 are emitted after
    MM1/softmax of tile i so the PE never waits on the softmax chain.
"""

import os

import numpy as np

import concourse.bacc as bacc
import concourse.bass as bass
import concourse.mybir as mybir
import concourse.tile as tile
from concourse.bass_utils import run_bass_kernel_spmd

F32 = mybir.dt.float32
F32R = mybir.dt.float32r
AF = mybir.ActivationFunctionType
ALU = mybir.AluOpType

P = 128
NEG_BIG = -1.0e12

N_CORES = 8
FULL_B, FULL_QL, FULL_KL, FULL_D = 8, 2048, 2048, 1024

# set to False to use plain fp32 matmuls (4x slower, exact) if fp32r
# accuracy turns out to be insufficient
USE_F32R = os.environ.get("ATTN_NO_F32R", "") == ""

LAST_RESULTS = None  # test harness introspection


MMDT = F32R if USE_F32R else F32


def _mm_cast(ap):
    return ap


def build_attention(QL, KL, D, with_mask2d):
    """Build the per-core SPMD Bass program.

    with_mask2d=False: inputs q,k,v,bias ([1,KL] additive row bias).
    with_mask2d=True : inputs q,k,v,mask2d ([QL,KL] additive mask =
                       attn_mask + pad bias, prepared on host).
    """
    inv_temp = float(1.0 / np.float32(np.sqrt(np.float32(D))))
    NQ = QL // P          # q tiles
    NKB = KL // P         # k blocks of 128
    ND = D // P           # d blocks of 128
    NKC = KL // 512       # k chunks of 512 (MM1 N dim)
    NDC = D // 512        # d chunks of 512 (MM2 N dim)
    assert QL % P == 0 and KL % 512 == 0 and D % 512 == 0

    nc = bacc.Bacc("TRN2", target_bir_lowering=False)

    qd = nc.declare_dram_parameter("q", [QL, D], F32, isOutput=False)
    identd = nc.declare_dram_parameter("ident", [P, P], F32, isOutput=False)
    kd = nc.declare_dram_parameter("k", [KL, D], F32, isOutput=False)
    vd = nc.declare_dram_parameter("v", [KL, D], F32, isOutput=False)
    if with_mask2d:
        md = nc.declare_dram_parameter("mask2d", [QL, KL], F32, isOutput=False)
    else:
        bd = nc.declare_dram_parameter("bias", [1, KL], F32, isOutput=False)
    ctxd = nc.declare_dram_parameter("ctx", [QL, D], F32, isOutput=True)
    wd = nc.declare_dram_parameter("w", [QL, KL], F32, isOutput=True)

    with tile.TileContext(nc) as tc:
        with (
            tc.tile_pool(name="const", bufs=1) as constp,
            tc.tile_pool(name="kv", bufs=1) as kvp,
            tc.tile_pool(name="psum_s", bufs=1, space="PSUM") as psum_s,
            tc.tile_pool(name="psum_c", bufs=1, space="PSUM") as psum_c,
            tc.tile_pool(name="psum_t", bufs=2, space="PSUM") as psum_t,
        ):
            ident = constp.tile([P, P], F32)
            nc.sync.dma_start(out=ident, in_=identd[:, :])

            if not with_mask2d:
                bias_b = constp.tile([P, KL], F32)

            # K^T resident: kt[:, dblk*KL + kcol], partition = d within block
            kt = kvp.tile([P, ND * KL], MMDT)
            # V resident (natural): vsb[:, kblk*D + dcol], partition = k in blk
            vsb = kvp.tile([P, NKB * D], MMDT)
            kt_r = kt.rearrange("p (b k) -> p b k", b=ND)

            with (
                tc.tile_pool(name="stage", bufs=5) as stagep,
                tc.tile_pool(name="qt", bufs=3) as qtp,
                tc.tile_pool(name="sw", bufs=3) as swp,
                tc.tile_pool(name="wt", bufs=1) as wtp,
                tc.tile_pool(name="cx", bufs=1) as cxp,
                tc.tile_pool(name="stat", bufs=2) as statp,
                tc.tile_pool(name="mf", bufs=1) as mfp,
            ):
                qt_tiles = {}
                sw_tiles = {}

                def prefetch_q(i):
                    qnat = stagep.tile([P, D], F32, tag="stage")
                    nc.sync.dma_start(out=qnat, in_=qd[i * P : (i + 1) * P, :])
                    qt = qtp.tile([P, D], MMDT, tag="qt")
                    for g in range(ND // 4):
                        tp = psum_t.tile([P, 512], F32, tag="tp")
                        for c in range(4):
                            dblk = g * 4 + c
                            nc.tensor.transpose(
                                tp[:, c * P : (c + 1) * P],
                                qnat[:, dblk * P : (dblk + 1) * P],
                                ident,
                            )
                        # fold the 1/sqrt(D) score scaling into Q here:
                        # exact for power-of-two D (sqrt(1024)=32)
                        nc.scalar.mul(qt[:, g * 512 : (g + 1) * 512], tp, inv_temp)
                    qt_tiles[i] = qt

                def mm1_chunk(i, s_ps, kc):
                    # one 512-wide k-chunk of S for q-tile i, accumulated
                    # over all 8 d-blocks (needs only k-tiles 4kc..4kc+3)
                    qt = qt_tiles[i]
                    for dblk in range(ND):
                        nc.tensor.matmul(
                            s_ps[:, kc * 512 : (kc + 1) * 512],
                            lhsT=qt[:, dblk * P : (dblk + 1) * P],
                            rhs=kt[
                                :, dblk * KL + kc * 512 : dblk * KL + (kc + 1) * 512
                            ],
                            start=(dblk == 0),
                            stop=(dblk == ND - 1),
                        )

                def softmax(i, s_ps):
                    sw = swp.tile([P, KL], F32, tag="sw")
                    if with_mask2d:
                        mf = mfp.tile([P, KL], F32, tag="mf")
                        nc.sync.dma_start(out=mf, in_=md[i * P : (i + 1) * P, :])
                        in1 = mf
                    else:
                        in1 = bias_b
                    # S_sbuf = S_psum + mask  (evacuates PSUM, applies mask)
                    nc.vector.tensor_add(sw, s_ps, in1)
                    negmax = statp.tile([P, 1], F32, tag="negmax")
                    nc.vector.reduce_max(
                        negmax, sw, axis=mybir.AxisListType.X, negate=True
                    )
                    rowsum = statp.tile([P, 1], F32, tag="rowsum")
                    nc.scalar.activation(
                        sw, sw, AF.Exp, bias=negmax, scale=1.0, accum_out=rowsum
                    )
                    recip = statp.tile([P, 1], F32, tag="recip")
                    nc.vector.reciprocal(recip, rowsum)
                    nc.vector.tensor_scalar_mul(sw, sw, recip)
                    nc.sync.dma_start(out=wd[i * P : (i + 1) * P, :], in_=sw)
                    sw_tiles[i] = sw

                def wt_mm2_ctx(i):
                    sw = sw_tiles.pop(i)
                    wt = wtp.tile([P, KL], MMDT, tag="wt")
                    for g in range(NKB // 4):
                        tp = psum_t.tile([P, 512], F32, tag="tp")
                        for c in range(4):
                            kblk = g * 4 + c
                            nc.tensor.transpose(
                                tp[:, c * P : (c + 1) * P],
                                sw[:, kblk * P : (kblk + 1) * P],
                                ident,
                            )
                        nc.scalar.copy(wt[:, g * 512 : (g + 1) * 512], tp)
                    c_ps = psum_c.tile([P, D], F32, tag="c")
                    for kblk in range(NKB):
                        lw = _mm_cast(wt[:, kblk * P : (kblk + 1) * P])
                        for dc in range(NDC):
                            nc.tensor.matmul(
                                c_ps[:, dc * 512 : (dc + 1) * 512],
                                lhsT=lw,
                                rhs=_mm_cast(
                                    vsb[:, kblk * D + dc * 512 : kblk * D + (dc + 1) * 512]
                                ),
                                start=(kblk == 0),
                                stop=(kblk == NKB - 1),
                            )
                    cx = cxp.tile([P, D], F32, tag="cx")
                    for dc in range(NDC):
                        nc.scalar.copy(
                            cx[:, dc * 512 : (dc + 1) * 512],
                            c_ps[:, dc * 512 : (dc + 1) * 512],
                        )
                    nc.sync.dma_start(out=ctxd[i * P : (i + 1) * P, :], in_=cx)

                # ---- prologue: Q(0), then K in groups of 4 tiles with
                # tile-0 MM1 chunks interleaved, then V ----
                def load_v(j0, j1):
                    for j in range(j0, min(j1, NKB)):
                        nc.sync.dma_start(
                            out=vsb[:, j * D : (j + 1) * D],
                            in_=vd[j * P : (j + 1) * P, :].bitcast(MMDT),
                        )

                prefetch_q(0)
                s_ps0 = psum_s.tile([P, KL], F32, tag="s")
                if not with_mask2d:
                    # replicate the [1, KL] pad-bias row to all partitions
                    nc.sync.dma_start(out=bias_b, in_=bd[0].partition_broadcast(P))
                for g in range(NKC):
                    for j in range(4 * g, 4 * g + 4):
                        knat = stagep.tile([P, D], F32, tag="stage")
                        nc.sync.dma_start(out=knat, in_=kd[j * P : (j + 1) * P, :])
                        for tg in range(ND // 4):
                            tp = psum_t.tile([P, 512], F32, tag="tp")
                            for c in range(4):
                                dblk = tg * 4 + c
                                nc.tensor.transpose(
                                    tp[:, c * P : (c + 1) * P],
                                    knat[:, dblk * P : (dblk + 1) * P],
                                    ident,
                                )
                            nc.scalar.copy(
                                kt_r[:, tg * 4 : tg * 4 + 4, j * P : (j + 1) * P],
                                tp.rearrange("p (b k) -> p b k", b=4),
                            )
                    mm1_chunk(0, s_ps0, g)
                if NQ > 1:
                    prefetch_q(1)
                load_v(0, 6)
                if NQ > 2:
                    prefetch_q(2)
                load_v(6, 12)
                softmax(0, s_ps0)

                DEPTH = 2
                for i in range(1, NQ):
                    if i == 1:
                        load_v(12, NKB)
                    if i + 2 < NQ:
                        prefetch_q(i + 2)
                    s_ps = psum_s.tile([P, KL], F32, tag="s")
                    for kc in range(NKC):
                        mm1_chunk(i, s_ps, kc)
                    softmax(i, s_ps)
                    if i >= DEPTH:
                        wt_mm2_ctx(i - DEPTH)
                for i in range(max(NQ - DEPTH, 0), NQ):
                    wt_mm2_ctx(i)

    nc.compile()
    return nc


_CACHE = {}


def _get_program(QL, KL, D, with_mask2d):
    key = (QL, KL, D, with_mask2d)
    if key not in _CACHE:
        _CACHE[key] = build_attention(QL, KL, D, with_mask2d)
    return _CACHE[key]


def kernel(Q, K, V, attn_mask, key_pad_mask):
    global LAST_RESULTS
    Q = np.ascontiguousarray(np.asarray(Q, dtype=np.float32))
    K = np.ascontiguousarray(np.asarray(K, dtype=np.float32))
    V = np.ascontiguousarray(np.asarray(V, dtype=np.float32))
    attn_mask = np.asarray(attn_mask, dtype=np.float32)
    pad = np.asarray(key_pad_mask).astype(bool)

    B, QL, D = Q.shape
    KL = K.shape[1]
    assert B == N_CORES, f"expected batch {N_CORES}, got {B}"

    with_mask2d = bool(np.any(attn_mask))
    nc = _get_program(QL, KL, D, with_mask2d)

    bias = np.where(pad, np.float32(NEG_BIG), np.float32(0.0)).astype(np.float32)
    ident_np = np.eye(P, dtype=np.float32)
    in_maps = []
    for b in range(B):
        m = {"q": Q[b], "k": K[b], "v": V[b], "ident": ident_np}
        if with_mask2d:
            m["mask2d"] = np.ascontiguousarray(
                attn_mask + bias[b][None, :], dtype=np.float32
            )
        else:
            m["bias"] = np.ascontiguousarray(bias[b][None, :])
        in_maps.append(m)

    res = run_bass_kernel_spmd(nc, in_maps, core_ids=list(range(N_CORES)))
    LAST_RESULTS = res

    ctx = np.stack([res.results[b]["ctx"] for b in range(B)])
    w = np.stack([res.results[b]["w"] for b in range(B)])
    return ctx, w
